# revision 28
# baseline (speedup 1.0000x reference)
"""Causal self-attention (B=4, T=2048, D=1024, H=16) on 8 TRN2 NeuronCores.

Sharding: tensor-parallel over 4 head-groups x data-parallel over 2 batch-groups.
Core c handles batches [2*(c//4), 2*(c//4)+2) and heads [4*(c%4), 4*(c%4)+4).
Each core computes a partial output projection (its 256 feature rows of W_proj);
the host sums the 4 head-group partials per batch group (f32 partials).

v3: single software-pipelined stream over 8 blocks (2 batches x 4 q-blocks).
 - x^T is pre-transposed AND pre-cast to bf16 on the HOST (xt dram tensor
   [NB, NDK, 128, T]), so there are no DMA transposes at all: plain 1KB-row
   DMAs, first matmul issues ~5us in.
 - Block m = (b, j): S(j) for both head pairs, then proj(m-1), then the
   QKV phase-A chunk for block m+1, then PV(j). Phase-A/proj matmuls are
   interleaved between S tiles as PE fill while the activation engine
   (the near-bottleneck, ~190us of exp) drains the 2-slot psS pool.
 - PSUM: psS 2x[128,1024] (4 banks) + ONE shared 4-slot [128,512] pool
   (psB) used round-robin by psQK/psV (phase A), psY (PV+denominator),
   and psO (projection) - 8 banks total.
 - The projection result is staged per-psO-tile as bf16 (DVE copy) and
   DMA'd out; host sums bf16 partials in f64.
 - exp: head pair packed [off:1024-off] (h1 shifted down by off on
   diagonal tiles) so the activation covers exactly the useful columns.
 - PV uses the 65-column stationary (ones col 0 + V cols 64:128) so the
   softmax denominator accumulates in psY row 0 for free; division is
   DVE reciprocal -> gpsimd partition_broadcast -> DVE multiply.
"""
import functools
from contextlib import ExitStack

import numpy as np
import ml_dtypes

import concourse.bacc as bacc
import concourse.tile as tile
import concourse.mybir as mybir
from concourse.bass_utils import run_bass_kernel_spmd

F32 = mybir.dt.float32
BF16 = mybir.dt.bfloat16
EXP = mybir.ActivationFunctionType.Exp

B, T, D, H, HD = 4, 2048, 1024, 16, 64
NB, NH = 2, 4            # batches / heads per core
DL = NH * HD             # local feature dim (256)
NC = 8
WCOL = 768               # per-dk weight columns: Q(256) K(256) V(256) packed
NT5 = T // 512           # 4  (512-token q blocks)
NTT = T // 128           # 16 (128-token key tiles)
NDK = D // 128           # 8  (feature chunks of input dim)


@functools.lru_cache(maxsize=1)
def build():
    nc = bacc.Bacc("TRN2", target_bir_lowering=False, debug=False, num_devices=NC)
    xt_d = nc.dram_tensor("xt", [NB, NDK, 128, T], BF16, kind="ExternalInput").ap()
    wqkv_d = nc.dram_tensor("wqkv", [D, WCOL], BF16, kind="ExternalInput").ap()
    wproj_d = nc.dram_tensor("wproj", [DL, D], BF16, kind="ExternalInput").ap()
    tri_d = nc.dram_tensor("tri", [128, 128], BF16, kind="ExternalInput").ap()
    out_d = nc.dram_tensor("out", [NB, T, D], BF16, kind="ExternalOutput").ap()

    with tile.TileContext(nc) as tc, ExitStack() as ctx:
        const = ctx.enter_context(tc.tile_pool(name="const", bufs=1))
        wpool = ctx.enter_context(tc.tile_pool(name="w", bufs=1))
        xtp = ctx.enter_context(tc.tile_pool(name="xt", bufs=5))
        actv = ctx.enter_context(tc.tile_pool(name="actv", bufs=1))
        pP = ctx.enter_context(tc.tile_pool(name="pP", bufs=33))
        ytp = ctx.enter_context(tc.tile_pool(name="ytp", bufs=2))
        rcp = ctx.enter_context(tc.tile_pool(name="rcp", bufs=2))
        ostp = ctx.enter_context(tc.tile_pool(name="ostp", bufs=3))
        psS_pool = ctx.enter_context(tc.tile_pool(name="psS", bufs=2, space="PSUM"))
        psB = ctx.enter_context(tc.tile_pool(name="psB", bufs=4, space="PSUM"))

        # scalar-queue DMA order = criticality: w Q-cols (first matmul
        # chain), K-cols, V-cols, tri (first diagonal mask ~20us), wp
        # (first projection ~40us). Startup is HBM-bound across all 8
        # cores, so the first chain's deps are kept minimal.
        w_sb = wpool.tile([128, NDK * WCOL], BF16)
        w3 = w_sb[:].rearrange("p (a q c) -> p a q c", a=NDK, q=3)
        wq_d3 = wqkv_d.rearrange("(a p) (q c) -> p a q c", p=128, q=3)
        # Q cols split in dk halves so the first matmul chain gates on
        # only 0.25MB; startup is HBM-bound across all 8 cores.
        nc.scalar.dma_start(w3[:, 0:4, 0], wq_d3[:, 0:4, 0])
        nc.scalar.dma_start(w3[:, 4:8, 0], wq_d3[:, 4:8, 0])
        for q3 in (1, 2):
            nc.scalar.dma_start(w3[:, :, q3], wq_d3[:, :, q3])
        tri = const.tile([128, 128], BF16)
        nc.scalar.dma_start(tri[:], tri_d)
        wp_sb = wpool.tile([128, 2 * D], BF16)
        nc.scalar.dma_start(
            wp_sb[:].rearrange("p (a c) -> p a c", a=2),
            wproj_d.rearrange("(a p) c -> p a c", p=128))

        # x^T chunks: xc[(b,t5)][p, dk, t'] = x[b, 512*t5+t', dk*128+p].
        # Plain DMA from the host-pretransposed layout; 1KB rows. Issued
        # just-in-time (chunk m+2 during block m) so the first chunk + w
        # have the full HBM bandwidth at startup.
        xcs = {}

        def xc_dma(ch):
            b, t5 = ch
            xc = xtp.tile([128, NDK, 512], BF16, tag="xc", name=f"xc{b}{t5}")
            src = xt_d[b, :, :, 512 * t5:512 * (t5 + 1)].rearrange("a p t -> p a t")
            if ch == (0, 0):   # split so the first matmuls gate on 0.5MB
                nc.sync.dma_start(xc[:, 0:4], src[:, 0:4])
                nc.sync.dma_start(xc[:, 4:8], src[:, 4:8])
            else:
                nc.sync.dma_start(xc[:], src)
            xcs[(b, t5)] = xc

        blocks = [(b, j) for b in range(NB) for j in range(NT5)]
        # chunks computed as fills during each block (feeding block m+1)
        chunk_fills = [[(0, 1)], [(0, 2)], [(0, 3)], [(1, 0)],
                       [(1, 1)], [(1, 2)], [(1, 3)], []]
        xc_dma((0, 0))
        for ch in chunk_fills[0] + chunk_fills[1]:
            xc_dma(ch)

        # V blocks per (key-tile ti, head h): 128 cols at (ti*NH+h)*128;
        # col 0 = ones (denominator lands in psY row 0 where
        # reciprocal_approx_fast works), cols 64:128 = V, 1:64 = zeros.
        vs, v128s, qts, kts = [], [], [], []
        for b in range(NB):
            v_sb = actv.tile([128, NTT * NH * 128], BF16, tag=f"v{b}", name=f"v{b}")
            v128 = v_sb[:].rearrange("p (n c) -> p n c", c=128)
            # only cols 0:64 of each 128-block are ever read beyond the V
            # data: col 0 = ones, 1:64 = zeros. b=0 on DVE (ready early),
            # b=1 on gpsimd.
            eng = nc.vector if b == 0 else nc.gpsimd
            eng.memset(v128[:, :, 1:64], 0.0)
            eng.memset(v128[:, :, 0:1], 1.0)
            vs.append(v_sb)
            v128s.append(v128)
            qts.append([actv.tile([128, T], BF16, tag=f"qt{b}{cc}", name=f"qt{b}{cc}")
                        for cc in range(2)])
            kts.append([actv.tile([128, T], BF16, tag=f"kt{b}{cc}", name=f"kt{b}{cc}")
                        for cc in range(2)])

        def phA_groups(b, t5):
            """Phase-A fill groups for chunk (b, t5): 2 QT + 2 KT + 4 V."""
            xc = xcs[(b, t5)]
            ts = slice(512 * t5, 512 * (t5 + 1))

            def qk(cc, base, dst):
                def emit():
                    ps = psB.tile([128, 512], F32, tag="b5", name="psQK")
                    for dk in range(NDK):
                        nc.tensor.matmul(
                            ps[:],
                            w_sb[:, dk * WCOL + base + cc * 128:
                                 dk * WCOL + base + cc * 128 + 128],
                            xc[:, dk, :],
                            start=(dk == 0), stop=(dk == NDK - 1))
                    nc.vector.tensor_copy(dst[cc][:, ts], ps[:])
                return emit

            def vv(tt):
                def emit():
                    ps = psB.tile([128, 512], F32, tag="b5", name="psV")
                    for dk in range(NDK):
                        nc.tensor.matmul(
                            ps[:, 0:256],
                            xc[:, dk, 128 * tt:128 * tt + 128],
                            w_sb[:, dk * WCOL + 512:dk * WCOL + 768],
                            start=(dk == 0), stop=(dk == NDK - 1))
                    ti = t5 * 4 + tt
                    nc.vector.tensor_copy(
                        v128s[b][:, ti * NH:(ti + 1) * NH, 64:128],
                        ps[:, 0:256].rearrange("p (n c) -> p n c", c=64))
                return emit

            return ([qk(cc, 0, qts[b]) for cc in range(2)]
                    + [qk(cc, 256, kts[b]) for cc in range(2)]
                    + [vv(tt) for tt in range(4)])

        def proj_groups(b, j, yts):
            """Projection fill groups for q-block (b, j): 8 psO tiles,
            each 2 matmuls + a direct PSUM->DRAM f32 DMA."""
            def po(tt, nn2):
                def emit():
                    ps = psB.tile([128, 512], F32, tag="b5", name="psO")
                    for ff in range(2):
                        nc.tensor.matmul(
                            ps[:],
                            yts[ff][:, 128 * tt:128 * tt + 128],
                            wp_sb[:, ff * D + 512 * nn2:ff * D + 512 * nn2 + 512],
                            start=(ff == 0), stop=(ff == 1))
                    ost = ostp.tile([128, 512], BF16, tag="o", name="ost")
                    nc.vector.tensor_copy(ost[:], ps[:])
                    nc.sync.dma_start(
                        out_d[b, 512 * j + 128 * tt:512 * j + 128 * tt + 128,
                              512 * nn2:512 * nn2 + 512],
                        ost[:])
                return emit
            return [po(tt, nn2) for tt in range(4) for nn2 in range(2)]

        prev = None   # (b, j, yts) of previous block
        carry = []    # proj fill groups deferred into the next block
        for m, (b, j) in enumerate(blocks):
            nk = 4 * j + 4
            offs = [128 * (i - 4 * j) if i > 4 * j else 0 for i in range(nk)]

            if m + 2 < len(blocks):   # two blocks of DMA lead time
                for ch in chunk_fills[m + 2]:
                    xc_dma(ch)
            # fill queue: phase A chunks first (they must land within this
            # block - the next block's S reads them), then carried-over and
            # fresh projection groups (those may slide one block further).
            phA_fills = []
            for ch in chunk_fills[m]:
                phA_fills += phA_groups(*ch)
            fills = phA_fills + carry
            if prev is not None:
                fills += proj_groups(*prev)
            if m == 0:
                # prologue: phase A chunk 0 emitted before anything else
                for g in phA_groups(0, 0):
                    g()

            # ---- S + exp for both head pairs, fills interleaved ----
            Ps = {}
            fi = 0
            for hp in range(2):
                qth, kth = qts[b][hp], kts[b][hp]
                for i in range(nk):
                    off = offs[i]
                    psS = psS_pool.tile([128, 1024], F32, tag="s", name="psS")
                    P = pP.tile([128, 1024], BF16, tag="p", name="P")
                    Ps[(hp, i)] = P
                    nc.tensor.matmul(
                        psS[:, off:512],
                        kth[0:64, 128 * i:128 * i + 128],
                        qth[0:64, 512 * j + off:512 * (j + 1)],
                        start=True, stop=True)
                    nc.tensor.matmul(
                        psS[:, 512:1024 - off],
                        kth[64:128, 128 * i:128 * i + 128],
                        qth[64:128, 512 * j + off:512 * (j + 1)],
                        start=True, stop=True)
                    nc.scalar.activation(
                        P[:, off:1024 - off], psS[:, off:1024 - off], EXP,
                        scale=0.125)
                    if i >= 4 * j:  # diagonal: causal triangle on both heads
                        nc.vector.tensor_mul(
                            P[:, off:off + 128], P[:, off:off + 128], tri[:])
                        nc.vector.tensor_mul(
                            P[:, 512:640], P[:, 512:640], tri[:])
                    # interleave fills in pairs every 2nd S tile: each
                    # row-tiled<->full-array transition exposes ~120ns of
                    # LDWEIGHTS, so group the S pairs to amortize it
                    if i % 2 == 1:
                        for g in fills[fi:fi + 2]:
                            g()
                        fi += 2
            # flush phase-A and carried fills now; let up to 4 fresh proj
            # groups slide into the next block (extra PE fill for the
            # ACT-heavy j=3 blocks). Flush everything at the last block.
            n_mand = len(phA_fills) + len(carry)
            limit = len(fills) if m == len(blocks) - 1 else max(
                n_mand, len(fills) - 4)
            while fi < limit:
                fills[fi]()
                fi += 1
            carry = fills[fi:]

            # ---- PV + normalize ----
            yt = [ytp.tile([128, 512], BF16, tag=f"yt{ff}", name=f"yt{ff}")
                  for ff in range(2)]
            for hp in range(2):
                for h01 in range(2):
                    h = 2 * hp + h01
                    psY = psB.tile([128, 512], F32, tag="b5", name="psY")
                    for i in range(nk):
                        off = offs[i]
                        mv = (Ps[(hp, i)][:, off:512] if h01 == 0
                              else Ps[(hp, i)][:, 512:1024 - off])
                        nc.tensor.matmul(
                            psY[:, off:512],
                            vs[b][:, 512 * i + 128 * h:512 * i + 128 * h + 128],
                            mv,
                            start=(i == 0), stop=(i == nk - 1))
                    rc = rcp.tile([1, 512], F32, tag="rc", name="rc")
                    nc.vector.reciprocal_approx_fast(rc[:], psY[0:1, :])
                    rb = rcp.tile([128, 512], F32, tag="rb", name="rb")
                    nc.gpsimd.partition_broadcast(rb[:], rc[:])
                    nc.vector.tensor_mul(
                        yt[hp][64 * h01:64 * h01 + 64, :],
                        psY[64:128, :], rb[64:128, :])
            prev = (b, j, yt)

        for g in proj_groups(*prev):   # epilogue: last block's projection
            g()

    nc.compile()
    return nc


def make_in_maps(x, W_qkv, W_proj):
    tri = np.triu(np.ones((128, 128), dtype=np.float32)).astype(ml_dtypes.bfloat16)
    xts = []
    for bg in range(2):
        xb = np.ascontiguousarray(x[2 * bg:2 * bg + 2]).astype(ml_dtypes.bfloat16)
        # [2, T, D] -> [2, NDK, 128, T]
        xts.append(np.ascontiguousarray(
            xb.reshape(NB, T, NDK, 128).transpose(0, 2, 3, 1)))
    in_maps = []
    for c in range(NC):
        bg, hg = c // 4, c % 4
        wq = np.concatenate(
            [W_qkv[:, 256 * hg:256 * hg + 256],
             W_qkv[:, 1024 + 256 * hg:1024 + 256 * hg + 256],
             W_qkv[:, 2048 + 256 * hg:2048 + 256 * hg + 256]], axis=1)
        in_maps.append({
            "xt": xts[bg],
            "wqkv": wq.astype(ml_dtypes.bfloat16),
            "wproj": W_proj[256 * hg:256 * hg + 256, :].astype(ml_dtypes.bfloat16),
            "tri": tri,
        })
    return in_maps


def kernel(x, W_qkv, W_proj):
    x = np.asarray(x, dtype=np.float32)
    W_qkv = np.asarray(W_qkv, dtype=np.float32)
    W_proj = np.asarray(W_proj, dtype=np.float32)
    nc = build()
    res = run_bass_kernel_spmd(nc, make_in_maps(x, W_qkv, W_proj), list(range(NC)))
    out = np.zeros((B, T, D), dtype=np.float64)
    for c in range(NC):
        bg = c // 4
        out[2 * bg:2 * bg + 2] += res.results[c]["out"].astype(np.float64)
    return out.astype(np.float32)


# revision 30
# speedup vs baseline: 1.0206x; 1.0206x over previous
"""Causal self-attention (B=4, T=2048, D=1024, H=16) on 8 TRN2 NeuronCores.

Sharding: tensor-parallel over 4 head-groups x data-parallel over 2 batch-groups.
Core c handles batches [2*(c//4), 2*(c//4)+2) and heads [4*(c%4), 4*(c%4)+4).
Each core computes a partial output projection (its 256 feature rows of W_proj);
the host sums the 4 head-group partials per batch group (f32 partials).

v3: single software-pipelined stream over 8 blocks (2 batches x 4 q-blocks).
 - x^T is pre-transposed AND pre-cast to bf16 on the HOST (xt dram tensor
   [NB, NDK, 128, T]), so there are no DMA transposes at all: plain 1KB-row
   DMAs, first matmul issues ~5us in.
 - Block m = (b, j): S(j) for both head pairs, then proj(m-1), then the
   QKV phase-A chunk for block m+1, then PV(j). Phase-A/proj matmuls are
   interleaved between S tiles as PE fill while the activation engine
   (the near-bottleneck, ~190us of exp) drains the 2-slot psS pool.
 - PSUM: psS 2x[128,1024] (4 banks) + ONE shared 4-slot [128,512] pool
   (psB) used round-robin by psQK/psV (phase A), psY (PV+denominator),
   and psO (projection) - 8 banks total.
 - The projection result is staged per-psO-tile as bf16 (DVE copy) and
   DMA'd out; host sums bf16 partials in f64.
 - exp: head pair packed [off:1024-off] (h1 shifted down by off on
   diagonal tiles) so the activation covers exactly the useful columns.
 - PV uses the 65-column stationary (ones col 0 + V cols 64:128) so the
   softmax denominator accumulates in psY row 0 for free; division is
   DVE reciprocal -> gpsimd partition_broadcast -> DVE multiply.
"""
import functools
from contextlib import ExitStack

import numpy as np
import ml_dtypes

import concourse.bacc as bacc
import concourse.tile as tile
import concourse.mybir as mybir
from concourse.bass_utils import run_bass_kernel_spmd

F32 = mybir.dt.float32
BF16 = mybir.dt.bfloat16
EXP = mybir.ActivationFunctionType.Exp

B, T, D, H, HD = 4, 2048, 1024, 16, 64
NB, NH = 2, 4            # batches / heads per core
DL = NH * HD             # local feature dim (256)
NC = 8
WCOL = 768               # per-dk weight columns: Q(256) K(256) V(256) packed
NT5 = T // 512           # 4  (512-token q blocks)
NTT = T // 128           # 16 (128-token key tiles)
NDK = D // 128           # 8  (feature chunks of input dim)


@functools.lru_cache(maxsize=1)
def build():
    nc = bacc.Bacc("TRN2", target_bir_lowering=False, debug=False, num_devices=NC)
    xt_d = nc.dram_tensor("xt", [NB, NDK, 128, T], BF16, kind="ExternalInput").ap()
    wqkv_d = nc.dram_tensor("wqkv", [D, WCOL], BF16, kind="ExternalInput").ap()
    wproj_d = nc.dram_tensor("wproj", [DL, D], BF16, kind="ExternalInput").ap()
    tri_d = nc.dram_tensor("tri", [128, 128], BF16, kind="ExternalInput").ap()
    out_d = nc.dram_tensor("out", [NB, T, D], BF16, kind="ExternalOutput").ap()

    with tile.TileContext(nc) as tc, ExitStack() as ctx:
        const = ctx.enter_context(tc.tile_pool(name="const", bufs=1))
        wpool = ctx.enter_context(tc.tile_pool(name="w", bufs=1))
        xtp = ctx.enter_context(tc.tile_pool(name="xt", bufs=5))
        actv = ctx.enter_context(tc.tile_pool(name="actv", bufs=1))
        pP = ctx.enter_context(tc.tile_pool(name="pP", bufs=33))
        ytp = ctx.enter_context(tc.tile_pool(name="ytp", bufs=2))
        rcp = ctx.enter_context(tc.tile_pool(name="rcp", bufs=2))
        ostp = ctx.enter_context(tc.tile_pool(name="ostp", bufs=3))
        psS_pool = ctx.enter_context(tc.tile_pool(name="psS", bufs=2, space="PSUM"))
        psB = ctx.enter_context(tc.tile_pool(name="psB", bufs=4, space="PSUM"))

        # scalar-queue DMA order = criticality: w Q-cols (first matmul
        # chain), K-cols, V-cols, tri (first diagonal mask ~20us), wp
        # (first projection ~40us). Startup is HBM-bound across all 8
        # cores, so the first chain's deps are kept minimal.
        w_sb = wpool.tile([128, NDK * WCOL], BF16)
        w3 = w_sb[:].rearrange("p (a q c) -> p a q c", a=NDK, q=3)
        wq_d3 = wqkv_d.rearrange("(a p) (q c) -> p a q c", p=128, q=3)
        # Q/K cols split in dk halves and interleaved so the first QT
        # chain gates on 0.25MB and the KT chains don't stall ~3us on a
        # late monolithic K transfer; startup is HBM-bound on all 8 cores.
        nc.scalar.dma_start(w3[:, 0:4, 0], wq_d3[:, 0:4, 0])
        nc.scalar.dma_start(w3[:, 0:4, 1], wq_d3[:, 0:4, 1])
        nc.scalar.dma_start(w3[:, 4:8, 0], wq_d3[:, 4:8, 0])
        nc.scalar.dma_start(w3[:, 4:8, 1], wq_d3[:, 4:8, 1])
        nc.scalar.dma_start(w3[:, :, 2], wq_d3[:, :, 2])
        tri = const.tile([128, 128], BF16)
        nc.scalar.dma_start(tri[:], tri_d)
        wp_sb = wpool.tile([128, 2 * D], BF16)
        nc.scalar.dma_start(
            wp_sb[:].rearrange("p (a c) -> p a c", a=2),
            wproj_d.rearrange("(a p) c -> p a c", p=128))

        # x^T chunks: xc[(b,t5)][p, dk, t'] = x[b, 512*t5+t', dk*128+p].
        # Plain DMA from the host-pretransposed layout; 1KB rows. Issued
        # just-in-time (chunk m+2 during block m) so the first chunk + w
        # have the full HBM bandwidth at startup.
        xcs = {}

        def xc_dma(ch):
            b, t5 = ch
            xc = xtp.tile([128, NDK, 512], BF16, tag="xc", name=f"xc{b}{t5}")
            src = xt_d[b, :, :, 512 * t5:512 * (t5 + 1)].rearrange("a p t -> p a t")
            if ch == (0, 0):   # split so the first matmuls gate on 0.5MB
                nc.sync.dma_start(xc[:, 0:4], src[:, 0:4])
                nc.sync.dma_start(xc[:, 4:8], src[:, 4:8])
            else:
                nc.sync.dma_start(xc[:], src)
            xcs[(b, t5)] = xc

        blocks = [(b, j) for b in range(NB) for j in range(NT5)]
        # chunks computed as fills during each block (feeding block m+1)
        chunk_fills = [[(0, 1)], [(0, 2)], [(0, 3)], [(1, 0)],
                       [(1, 1)], [(1, 2)], [(1, 3)], []]
        xc_dma((0, 0))
        for ch in chunk_fills[0] + chunk_fills[1]:
            xc_dma(ch)

        # V blocks per (key-tile ti, head h): 128 cols at (ti*NH+h)*128;
        # col 0 = ones (denominator lands in psY row 0 where
        # reciprocal_approx_fast works), cols 64:128 = V, 1:64 = zeros.
        vs, v128s, qts, kts = [], [], [], []
        for b in range(NB):
            v_sb = actv.tile([128, NTT * NH * 128], BF16, tag=f"v{b}", name=f"v{b}")
            v128 = v_sb[:].rearrange("p (n c) -> p n c", c=128)
            # only cols 0:64 of each 128-block are ever read beyond the V
            # data: col 0 = ones, 1:64 = zeros. b=0 on DVE (ready early),
            # b=1 on gpsimd.
            eng = nc.vector if b == 0 else nc.gpsimd
            eng.memset(v128[:, :, 1:64], 0.0)
            eng.memset(v128[:, :, 0:1], 1.0)
            vs.append(v_sb)
            v128s.append(v128)
            qts.append([actv.tile([128, T], BF16, tag=f"qt{b}{cc}", name=f"qt{b}{cc}")
                        for cc in range(2)])
            kts.append([actv.tile([128, T], BF16, tag=f"kt{b}{cc}", name=f"kt{b}{cc}")
                        for cc in range(2)])

        def phA_groups(b, t5):
            """Phase-A fill groups for chunk (b, t5): 2 QT + 2 KT + 4 V."""
            xc = xcs[(b, t5)]
            ts = slice(512 * t5, 512 * (t5 + 1))

            def qk(cc, base, dst):
                def emit():
                    ps = psB.tile([128, 512], F32, tag="b5", name="psQK")
                    for dk in range(NDK):
                        nc.tensor.matmul(
                            ps[:],
                            w_sb[:, dk * WCOL + base + cc * 128:
                                 dk * WCOL + base + cc * 128 + 128],
                            xc[:, dk, :],
                            start=(dk == 0), stop=(dk == NDK - 1))
                    nc.vector.tensor_copy(dst[cc][:, ts], ps[:])
                return emit

            def vv(tt):
                def emit():
                    ps = psB.tile([128, 512], F32, tag="b5", name="psV")
                    for dk in range(NDK):
                        nc.tensor.matmul(
                            ps[:, 0:256],
                            xc[:, dk, 128 * tt:128 * tt + 128],
                            w_sb[:, dk * WCOL + 512:dk * WCOL + 768],
                            start=(dk == 0), stop=(dk == NDK - 1))
                    ti = t5 * 4 + tt
                    nc.vector.tensor_copy(
                        v128s[b][:, ti * NH:(ti + 1) * NH, 64:128],
                        ps[:, 0:256].rearrange("p (n c) -> p n c", c=64))
                return emit

            return ([qk(cc, 0, qts[b]) for cc in range(2)]
                    + [qk(cc, 256, kts[b]) for cc in range(2)]
                    + [vv(tt) for tt in range(4)])

        def proj_groups(b, j, yts):
            """Projection fill groups for q-block (b, j): 8 psO tiles,
            each 2 matmuls + a direct PSUM->DRAM f32 DMA."""
            def po(tt, nn2):
                def emit():
                    ps = psB.tile([128, 512], F32, tag="b5", name="psO")
                    for ff in range(2):
                        nc.tensor.matmul(
                            ps[:],
                            yts[ff][:, 128 * tt:128 * tt + 128],
                            wp_sb[:, ff * D + 512 * nn2:ff * D + 512 * nn2 + 512],
                            start=(ff == 0), stop=(ff == 1))
                    ost = ostp.tile([128, 512], BF16, tag="o", name="ost")
                    nc.vector.tensor_copy(ost[:], ps[:])
                    nc.sync.dma_start(
                        out_d[b, 512 * j + 128 * tt:512 * j + 128 * tt + 128,
                              512 * nn2:512 * nn2 + 512],
                        ost[:])
                return emit
            return [po(tt, nn2) for tt in range(4) for nn2 in range(2)]

        prev = None   # (b, j, yts) of previous block
        carry = []    # proj fill groups deferred into the next block
        for m, (b, j) in enumerate(blocks):
            nk = 4 * j + 4
            offs = [128 * (i - 4 * j) if i > 4 * j else 0 for i in range(nk)]

            # three blocks of DMA lead time mid-kernel (the sync queue
            # serializes descriptor processing behind psO out-DMAs), but
            # keep the prologue at 3 chunks so startup HBM load is flat.
            if m == 0:
                for ch in chunk_fills[2] + chunk_fills[3]:
                    xc_dma(ch)
            elif m + 3 < len(blocks):
                for ch in chunk_fills[m + 3]:
                    xc_dma(ch)
            # fill queue: phase A chunks first (they must land within this
            # block - the next block's S reads them), then carried-over and
            # fresh projection groups (those may slide one block further).
            phA_fills = []
            for ch in chunk_fills[m]:
                phA_fills += phA_groups(*ch)
            fills = phA_fills + carry
            if prev is not None:
                fills += proj_groups(*prev)
            if m == 0:
                # prologue: phase A chunk 0 emitted before anything else
                for g in phA_groups(0, 0):
                    g()

            # ---- S + exp for both head pairs, fills interleaved ----
            Ps = {}
            fi = 0
            for hp in range(2):
                qth, kth = qts[b][hp], kts[b][hp]
                for i in range(nk):
                    off = offs[i]
                    psS = psS_pool.tile([128, 1024], F32, tag="s", name="psS")
                    P = pP.tile([128, 1024], BF16, tag="p", name="P")
                    Ps[(hp, i)] = P
                    nc.tensor.matmul(
                        psS[:, off:512],
                        kth[0:64, 128 * i:128 * i + 128],
                        qth[0:64, 512 * j + off:512 * (j + 1)],
                        start=True, stop=True)
                    nc.tensor.matmul(
                        psS[:, 512:1024 - off],
                        kth[64:128, 128 * i:128 * i + 128],
                        qth[64:128, 512 * j + off:512 * (j + 1)],
                        start=True, stop=True)
                    nc.scalar.activation(
                        P[:, off:1024 - off], psS[:, off:1024 - off], EXP,
                        scale=0.125)
                    if i >= 4 * j:  # diagonal: causal triangle on both heads
                        nc.vector.tensor_mul(
                            P[:, off:off + 128], P[:, off:off + 128], tri[:])
                        nc.vector.tensor_mul(
                            P[:, 512:640], P[:, 512:640], tri[:])
                    # interleave fills in pairs every 2nd S tile: each
                    # row-tiled<->full-array transition exposes ~120ns of
                    # LDWEIGHTS, so group the S pairs to amortize it
                    if i % 2 == 1:
                        for g in fills[fi:fi + 2]:
                            g()
                        fi += 2
            # flush phase-A and carried fills now; let up to 4 fresh proj
            # groups slide into the next block (extra PE fill for the
            # ACT-heavy j=3 blocks). Flush everything at the last block.
            n_mand = len(phA_fills) + len(carry)
            limit = len(fills) if m == len(blocks) - 1 else max(
                n_mand, len(fills) - 4)
            while fi < limit:
                fills[fi]()
                fi += 1
            carry = fills[fi:]

            # ---- PV + normalize ----
            yt = [ytp.tile([128, 512], BF16, tag=f"yt{ff}", name=f"yt{ff}")
                  for ff in range(2)]
            for hp in range(2):
                for h01 in range(2):
                    h = 2 * hp + h01
                    psY = psB.tile([128, 512], F32, tag="b5", name="psY")
                    for i in range(nk):
                        off = offs[i]
                        mv = (Ps[(hp, i)][:, off:512] if h01 == 0
                              else Ps[(hp, i)][:, 512:1024 - off])
                        nc.tensor.matmul(
                            psY[:, off:512],
                            vs[b][:, 512 * i + 128 * h:512 * i + 128 * h + 128],
                            mv,
                            start=(i == 0), stop=(i == nk - 1))
                    rc = rcp.tile([1, 512], F32, tag="rc", name="rc")
                    nc.vector.reciprocal_approx_fast(rc[:], psY[0:1, :])
                    rb = rcp.tile([128, 512], F32, tag="rb", name="rb")
                    nc.gpsimd.partition_broadcast(rb[:], rc[:])
                    nc.vector.tensor_mul(
                        yt[hp][64 * h01:64 * h01 + 64, :],
                        psY[64:128, :], rb[64:128, :])
            prev = (b, j, yt)

        for g in proj_groups(*prev):   # epilogue: last block's projection
            g()

    nc.compile()
    return nc


def make_in_maps(x, W_qkv, W_proj):
    tri = np.triu(np.ones((128, 128), dtype=np.float32)).astype(ml_dtypes.bfloat16)
    xts = []
    for bg in range(2):
        xb = np.ascontiguousarray(x[2 * bg:2 * bg + 2]).astype(ml_dtypes.bfloat16)
        # [2, T, D] -> [2, NDK, 128, T]
        xts.append(np.ascontiguousarray(
            xb.reshape(NB, T, NDK, 128).transpose(0, 2, 3, 1)))
    in_maps = []
    for c in range(NC):
        bg, hg = c // 4, c % 4
        wq = np.concatenate(
            [W_qkv[:, 256 * hg:256 * hg + 256],
             W_qkv[:, 1024 + 256 * hg:1024 + 256 * hg + 256],
             W_qkv[:, 2048 + 256 * hg:2048 + 256 * hg + 256]], axis=1)
        in_maps.append({
            "xt": xts[bg],
            "wqkv": wq.astype(ml_dtypes.bfloat16),
            "wproj": W_proj[256 * hg:256 * hg + 256, :].astype(ml_dtypes.bfloat16),
            "tri": tri,
        })
    return in_maps


def kernel(x, W_qkv, W_proj):
    x = np.asarray(x, dtype=np.float32)
    W_qkv = np.asarray(W_qkv, dtype=np.float32)
    W_proj = np.asarray(W_proj, dtype=np.float32)
    nc = build()
    res = run_bass_kernel_spmd(nc, make_in_maps(x, W_qkv, W_proj), list(range(NC)))
    out = np.zeros((B, T, D), dtype=np.float64)
    for c in range(NC):
        bg = c // 4
        out[2 * bg:2 * bg + 2] += res.results[c]["out"].astype(np.float64)
    return out.astype(np.float32)


# revision 32
# speedup vs baseline: 1.0426x; 1.0215x over previous
"""Causal self-attention (B=4, T=2048, D=1024, H=16) on 8 TRN2 NeuronCores.

Sharding: tensor-parallel over 4 head-groups x data-parallel over 2 batch-groups.
Core c handles batches [2*(c//4), 2*(c//4)+2) and heads [4*(c%4), 4*(c%4)+4).
Each core computes a partial output projection (its 256 feature rows of W_proj);
the host sums the 4 head-group partials per batch group (f32 partials).

v3: single software-pipelined stream over 8 blocks (2 batches x 4 q-blocks).
 - x^T is pre-transposed AND pre-cast to bf16 on the HOST (xt dram tensor
   [NB, NDK, 128, T]), so there are no DMA transposes at all: plain 1KB-row
   DMAs, first matmul issues ~5us in.
 - Block m = (b, j): S(j) for both head pairs, then proj(m-1), then the
   QKV phase-A chunk for block m+1, then PV(j). Phase-A/proj matmuls are
   interleaved between S tiles as PE fill while the activation engine
   (the near-bottleneck, ~190us of exp) drains the 2-slot psS pool.
 - PSUM: psS 2x[128,1024] (4 banks) + ONE shared 4-slot [128,512] pool
   (psB) used round-robin by psQK/psV (phase A), psY (PV+denominator),
   and psO (projection) - 8 banks total.
 - The projection result is staged per-psO-tile as bf16 (DVE copy) and
   DMA'd out; host sums bf16 partials in f64.
 - exp: head pair packed [off:1024-off] (h1 shifted down by off on
   diagonal tiles) so the activation covers exactly the useful columns.
 - PV uses the 65-column stationary (ones col 0 + V cols 64:128) so the
   softmax denominator accumulates in psY row 0 for free; division is
   DVE reciprocal -> gpsimd partition_broadcast -> DVE multiply.
"""
import functools
from contextlib import ExitStack

import numpy as np
import ml_dtypes

import concourse.bacc as bacc
import concourse.tile as tile
import concourse.mybir as mybir
from concourse.bass_utils import run_bass_kernel_spmd

F32 = mybir.dt.float32
BF16 = mybir.dt.bfloat16
EXP = mybir.ActivationFunctionType.Exp

B, T, D, H, HD = 4, 2048, 1024, 16, 64
NB, NH = 2, 4            # batches / heads per core
DL = NH * HD             # local feature dim (256)
NC = 8
WCOL = 768               # per-dk weight columns: Q(256) K(256) V(256) packed
NT5 = T // 512           # 4  (512-token q blocks)
NTT = T // 128           # 16 (128-token key tiles)
NDK = D // 128           # 8  (feature chunks of input dim)


@functools.lru_cache(maxsize=1)
def build():
    nc = bacc.Bacc("TRN2", target_bir_lowering=False, debug=False, num_devices=NC)
    xt_d = nc.dram_tensor("xt", [NB, NDK, 128, T], BF16, kind="ExternalInput").ap()
    wqkv_d = nc.dram_tensor("wqkv", [D, WCOL], BF16, kind="ExternalInput").ap()
    wproj_d = nc.dram_tensor("wproj", [DL, D], BF16, kind="ExternalInput").ap()
    tri_d = nc.dram_tensor("tri", [128, 128], BF16, kind="ExternalInput").ap()
    out_d = nc.dram_tensor("out", [NB, T, D], BF16, kind="ExternalOutput").ap()

    with tile.TileContext(nc) as tc, ExitStack() as ctx:
        const = ctx.enter_context(tc.tile_pool(name="const", bufs=1))
        wpool = ctx.enter_context(tc.tile_pool(name="w", bufs=1))
        xtp = ctx.enter_context(tc.tile_pool(name="xt", bufs=5))
        actv = ctx.enter_context(tc.tile_pool(name="actv", bufs=1))
        pP = ctx.enter_context(tc.tile_pool(name="pP", bufs=33))
        ytp = ctx.enter_context(tc.tile_pool(name="ytp", bufs=2))
        rcp = ctx.enter_context(tc.tile_pool(name="rcp", bufs=2))
        ostp = ctx.enter_context(tc.tile_pool(name="ostp", bufs=3))
        psS_pool = ctx.enter_context(tc.tile_pool(name="psS", bufs=2, space="PSUM"))
        psB = ctx.enter_context(tc.tile_pool(name="psB", bufs=4, space="PSUM"))

        # scalar-queue DMA order = criticality: w Q-cols (first matmul
        # chain), K-cols, V-cols, tri (first diagonal mask ~20us), wp
        # (first projection ~40us). Startup is HBM-bound across all 8
        # cores, so the first chain's deps are kept minimal.
        w_sb = wpool.tile([128, NDK * WCOL], BF16)
        w3 = w_sb[:].rearrange("p (a q c) -> p a q c", a=NDK, q=3)
        wq_d3 = wqkv_d.rearrange("(a p) (q c) -> p a q c", p=128, q=3)
        # Q/K cols split in dk halves and interleaved so the first QT
        # chain gates on 0.25MB and the KT chains don't stall ~3us on a
        # late monolithic K transfer; startup is HBM-bound on all 8 cores.
        nc.scalar.dma_start(w3[:, 0:4, 0], wq_d3[:, 0:4, 0])
        nc.scalar.dma_start(w3[:, 0:4, 1], wq_d3[:, 0:4, 1])
        nc.scalar.dma_start(w3[:, 4:8, 0], wq_d3[:, 4:8, 0])
        nc.scalar.dma_start(w3[:, 4:8, 1], wq_d3[:, 4:8, 1])
        nc.scalar.dma_start(w3[:, :, 2], wq_d3[:, :, 2])
        tri = const.tile([128, 128], BF16)
        nc.scalar.dma_start(tri[:], tri_d)
        wp_sb = wpool.tile([128, 2 * D], BF16)
        nc.scalar.dma_start(
            wp_sb[:].rearrange("p (a c) -> p a c", a=2),
            wproj_d.rearrange("(a p) c -> p a c", p=128))

        # x^T chunks: xc[(b,t5)][p, dk, t'] = x[b, 512*t5+t', dk*128+p].
        # Plain DMA from the host-pretransposed layout; 1KB rows. Issued
        # just-in-time (chunk m+2 during block m) so the first chunk + w
        # have the full HBM bandwidth at startup.
        xcs = {}

        def xc_dma(ch):
            b, t5 = ch
            xc = xtp.tile([128, NDK, 512], BF16, tag="xc", name=f"xc{b}{t5}")
            src = xt_d[b, :, :, 512 * t5:512 * (t5 + 1)].rearrange("a p t -> p a t")
            if ch == (0, 0):   # split so the first matmuls gate on 0.5MB
                nc.sync.dma_start(xc[:, 0:4], src[:, 0:4])
                nc.sync.dma_start(xc[:, 4:8], src[:, 4:8])
            else:
                nc.sync.dma_start(xc[:], src)
            xcs[(b, t5)] = xc

        blocks = [(b, j) for b in range(NB) for j in range(NT5)]
        # chunks computed as fills during each block (feeding block m+1)
        chunk_fills = [[(0, 1)], [(0, 2)], [(0, 3)], [(1, 0)],
                       [(1, 1)], [(1, 2)], [(1, 3)], []]
        xc_dma((0, 0))
        for ch in chunk_fills[0] + chunk_fills[1]:
            xc_dma(ch)

        # V blocks per (key-tile ti, head h): 128 cols at (ti*NH+h)*128;
        # col 0 = ones (denominator lands in psY row 0 where
        # reciprocal_approx_fast works), cols 64:128 = V, 1:64 = zeros.
        vs, v128s, qts, kts = [], [], [], []
        for b in range(NB):
            v_sb = actv.tile([128, NTT * NH * 128], BF16, tag=f"v{b}", name=f"v{b}")
            v128 = v_sb[:].rearrange("p (n c) -> p n c", c=128)
            # only cols 0:64 of each 128-block are ever read beyond the V
            # data: col 0 = ones, 1:64 = zeros. b=0 on DVE (ready early),
            # b=1 on gpsimd.
            eng = nc.vector if b == 0 else nc.gpsimd
            eng.memset(v128[:, :, 1:64], 0.0)
            eng.memset(v128[:, :, 0:1], 1.0)
            vs.append(v_sb)
            v128s.append(v128)
            qts.append([actv.tile([128, T], BF16, tag=f"qt{b}{cc}", name=f"qt{b}{cc}")
                        for cc in range(2)])
            kts.append([actv.tile([128, T], BF16, tag=f"kt{b}{cc}", name=f"kt{b}{cc}")
                        for cc in range(2)])

        def phA_groups(b, t5):
            """Phase-A fill groups for chunk (b, t5): 2 QT + 2 KT + 4 V."""
            xc = xcs[(b, t5)]
            ts = slice(512 * t5, 512 * (t5 + 1))

            def qk(cc, base, dst):
                def emit():
                    ps = psB.tile([128, 512], F32, tag="b5", name="psQK")
                    for dk in range(NDK):
                        nc.tensor.matmul(
                            ps[:],
                            w_sb[:, dk * WCOL + base + cc * 128:
                                 dk * WCOL + base + cc * 128 + 128],
                            xc[:, dk, :],
                            start=(dk == 0), stop=(dk == NDK - 1))
                    nc.vector.tensor_copy(dst[cc][:, ts], ps[:])
                return emit

            def vv(tt):
                def emit():
                    ps = psB.tile([128, 512], F32, tag="b5", name="psV")
                    for dk in range(NDK):
                        nc.tensor.matmul(
                            ps[:, 0:256],
                            xc[:, dk, 128 * tt:128 * tt + 128],
                            w_sb[:, dk * WCOL + 512:dk * WCOL + 768],
                            start=(dk == 0), stop=(dk == NDK - 1))
                    ti = t5 * 4 + tt
                    nc.vector.tensor_copy(
                        v128s[b][:, ti * NH:(ti + 1) * NH, 64:128],
                        ps[:, 0:256].rearrange("p (n c) -> p n c", c=64))
                return emit

            return ([qk(cc, 0, qts[b]) for cc in range(2)]
                    + [qk(cc, 256, kts[b]) for cc in range(2)]
                    + [vv(tt) for tt in range(4)])

        def proj_groups(b, j, yts):
            """Projection fill groups for q-block (b, j): 8 psO tiles,
            each 2 matmuls + a direct PSUM->DRAM f32 DMA."""
            def po(tt, nn2):
                def emit():
                    ps = psB.tile([128, 512], F32, tag="b5", name="psO")
                    for ff in range(2):
                        nc.tensor.matmul(
                            ps[:],
                            yts[ff][:, 128 * tt:128 * tt + 128],
                            wp_sb[:, ff * D + 512 * nn2:ff * D + 512 * nn2 + 512],
                            start=(ff == 0), stop=(ff == 1))
                    ost = ostp.tile([128, 512], BF16, tag="o", name="ost")
                    nc.vector.tensor_copy(ost[:], ps[:])
                    nc.sync.dma_start(
                        out_d[b, 512 * j + 128 * tt:512 * j + 128 * tt + 128,
                              512 * nn2:512 * nn2 + 512],
                        ost[:])
                return emit
            return [po(tt, nn2) for tt in range(4) for nn2 in range(2)]

        prev = None   # (b, j, yts) of previous block
        carry = []    # proj fill groups deferred into the next block
        for m, (b, j) in enumerate(blocks):
            nk = 4 * j + 4
            offs = [128 * (i - 4 * j) if i > 4 * j else 0 for i in range(nk)]

            # three blocks of DMA lead time mid-kernel (the sync queue
            # serializes descriptor processing behind psO out-DMAs), but
            # keep the prologue at 3 chunks so startup HBM load is flat.
            if m == 0:
                for ch in chunk_fills[2] + chunk_fills[3]:
                    xc_dma(ch)
            elif m + 3 < len(blocks):
                for ch in chunk_fills[m + 3]:
                    xc_dma(ch)
            # fill queue: phase A chunks first (they must land within this
            # block - the next block's S reads them), then carried-over and
            # fresh projection groups (those may slide one block further).
            phA_fills = []
            phA_defer = []
            for ch in chunk_fills[m]:
                gs = phA_groups(*ch)
                if m == len(blocks) - 2:
                    # the V groups of the last chunk are only read by the
                    # final block's late PV tiles - slide them there as
                    # extra PE fill (block 7 otherwise starves on ACT)
                    phA_fills += gs[:4]
                    phA_defer = gs[4:]
                else:
                    phA_fills += gs
            fills = phA_fills + carry
            if prev is not None:
                fills += proj_groups(*prev)
            if m == 0:
                # prologue: phase A chunk 0 emitted before anything else
                for g in phA_groups(0, 0):
                    g()

            # ---- S + exp for both head pairs, fills interleaved ----
            Ps = {}
            fi = 0
            for hp in range(2):
                qth, kth = qts[b][hp], kts[b][hp]
                for i in range(nk):
                    off = offs[i]
                    psS = psS_pool.tile([128, 1024], F32, tag="s", name="psS")
                    P = pP.tile([128, 1024], BF16, tag="p", name="P")
                    Ps[(hp, i)] = P
                    nc.tensor.matmul(
                        psS[:, off:512],
                        kth[0:64, 128 * i:128 * i + 128],
                        qth[0:64, 512 * j + off:512 * (j + 1)],
                        start=True, stop=True)
                    nc.tensor.matmul(
                        psS[:, 512:1024 - off],
                        kth[64:128, 128 * i:128 * i + 128],
                        qth[64:128, 512 * j + off:512 * (j + 1)],
                        start=True, stop=True)
                    nc.scalar.activation(
                        P[:, off:1024 - off], psS[:, off:1024 - off], EXP,
                        scale=0.125)
                    if i >= 4 * j:  # diagonal: causal triangle on both heads
                        nc.vector.tensor_mul(
                            P[:, off:off + 128], P[:, off:off + 128], tri[:])
                        nc.vector.tensor_mul(
                            P[:, 512:640], P[:, 512:640], tri[:])
                    # interleave fills in pairs every 2nd S tile: each
                    # row-tiled<->full-array transition exposes ~120ns of
                    # LDWEIGHTS, so group the S pairs to amortize it
                    if i % 2 == 1:
                        for g in fills[fi:fi + 2]:
                            g()
                        fi += 2
            # flush phase-A and carried fills now; let up to 4 fresh proj
            # groups slide into the next block (extra PE fill for the
            # ACT-heavy j=3 blocks). Flush everything at the last block.
            n_mand = len(phA_fills) + len(carry)
            limit = len(fills) if m == len(blocks) - 1 else max(
                n_mand, len(fills) - 4)
            while fi < limit:
                fills[fi]()
                fi += 1
            carry = phA_defer + fills[fi:]

            # ---- PV + normalize ----
            yt = [ytp.tile([128, 512], BF16, tag=f"yt{ff}", name=f"yt{ff}")
                  for ff in range(2)]
            for hp in range(2):
                for h01 in range(2):
                    h = 2 * hp + h01
                    psY = psB.tile([128, 512], F32, tag="b5", name="psY")
                    for i in range(nk):
                        off = offs[i]
                        mv = (Ps[(hp, i)][:, off:512] if h01 == 0
                              else Ps[(hp, i)][:, 512:1024 - off])
                        nc.tensor.matmul(
                            psY[:, off:512],
                            vs[b][:, 512 * i + 128 * h:512 * i + 128 * h + 128],
                            mv,
                            start=(i == 0), stop=(i == nk - 1))
                    rc = rcp.tile([1, 512], F32, tag="rc", name="rc")
                    nc.vector.reciprocal_approx_fast(rc[:], psY[0:1, :])
                    rb = rcp.tile([128, 512], F32, tag="rb", name="rb")
                    nc.gpsimd.partition_broadcast(rb[:], rc[:])
                    nc.vector.tensor_mul(
                        yt[hp][64 * h01:64 * h01 + 64, :],
                        psY[64:128, :], rb[64:128, :])
            prev = (b, j, yt)

        for g in proj_groups(*prev):   # epilogue: last block's projection
            g()

    nc.compile()
    return nc


def make_in_maps(x, W_qkv, W_proj):
    tri = np.triu(np.ones((128, 128), dtype=np.float32)).astype(ml_dtypes.bfloat16)
    xts = []
    for bg in range(2):
        xb = np.ascontiguousarray(x[2 * bg:2 * bg + 2]).astype(ml_dtypes.bfloat16)
        # [2, T, D] -> [2, NDK, 128, T]
        xts.append(np.ascontiguousarray(
            xb.reshape(NB, T, NDK, 128).transpose(0, 2, 3, 1)))
    in_maps = []
    for c in range(NC):
        bg, hg = c // 4, c % 4
        wq = np.concatenate(
            [W_qkv[:, 256 * hg:256 * hg + 256],
             W_qkv[:, 1024 + 256 * hg:1024 + 256 * hg + 256],
             W_qkv[:, 2048 + 256 * hg:2048 + 256 * hg + 256]], axis=1)
        in_maps.append({
            "xt": xts[bg],
            "wqkv": wq.astype(ml_dtypes.bfloat16),
            "wproj": W_proj[256 * hg:256 * hg + 256, :].astype(ml_dtypes.bfloat16),
            "tri": tri,
        })
    return in_maps


def kernel(x, W_qkv, W_proj):
    x = np.asarray(x, dtype=np.float32)
    W_qkv = np.asarray(W_qkv, dtype=np.float32)
    W_proj = np.asarray(W_proj, dtype=np.float32)
    nc = build()
    res = run_bass_kernel_spmd(nc, make_in_maps(x, W_qkv, W_proj), list(range(NC)))
    out = np.zeros((B, T, D), dtype=np.float64)
    for c in range(NC):
        bg = c // 4
        out[2 * bg:2 * bg + 2] += res.results[c]["out"].astype(np.float64)
    return out.astype(np.float32)
